# revision 1
# baseline (speedup 1.0000x reference)
"""GQA kernel for Trainium2, sharded over 8 NeuronCores.

Sharding: data-parallel over batch (2) x tensor-parallel over kv_heads (4).
Core c = b*4 + h computes the full attention output partial
    Y_bh = softmax(causal((Q_b @ Wq_eff_h) @ (K_b @ Wk_h)^T / sqrt(dk))) @ (V_b @ Wv_h) @ Wo_h
and the host sums the 4 head partials per batch (the "all-reduce after Wo").

The GQA group-sum-before-softmax quirk folds into the weights:
    scores_h = sum_g (Q Wq_{g,h}) (K Wk_h)^T = (Q [sum_g Wq_{g,h}]) (K Wk_h)^T
so Wq_eff_h = sum_g Wq[:, (g*KV+h)*dk : ...] and each core runs standard attention.

Device schedule (single in-order pass, DMA-wire is the binding resource):
  1. k-projection, q-projection (activation rows streamed as 512KB chunks)
  2. v-projection DMA/matmuls interleaved with ALL attention score work
     (S^T tiles -> exp -> causal mask -> ones-matmul row sums); the exp'd
     P^T tiles stay resident in SBUF (40KB/partition) so the PV matmuls
     can wait for v without stalling the scores.
  3. v transposes (PE), then per query-chunk: PV accumulation, softmax
     normalization folded into the O^T eviction, Y = O @ Wo, fp16 out.

Layouts (SBUF partition dim first): qT/kT/vT (dk=128, L) fp16; S^T tiles
(Lk_t=128, Lq=512) fp32 psum; row sums via ones-matmul (result replicated
across partitions == the free-dim broadcast needed to normalize O^T).
"""
import sys
sys.path.insert(0, '/opt/trn_rl_repo')
import math
import numpy as np

import concourse.bass as bass
import concourse.mybir as mybir
import concourse.tile as tile
from concourse import bacc
from concourse import bass_utils
from concourse.masks import make_identity

FP32 = mybir.dt.float32
FP16 = mybir.dt.float16

B, L, D = 2, 2048, 2048
Q_HEADS, KV_HEADS, DK, DV = 16, 4, 128, 128
GROUPS = Q_HEADS // KV_HEADS
P = 128
CH = 512                 # Lq chunk width
NJ = L // CH             # 4 query chunks
NDC = D // P             # 16 contraction chunks
NLK = L // P             # 16 key tiles
SCALE = 1.0 / math.sqrt(DK)
EBIAS = -8.0 * math.log(2.0)   # exp output scaled by 2^-8; cancels in softmax
YDT = FP16               # partial-output dtype (host accumulates in fp32)
YNP = np.float16

# flattened score work items (j, c), j-major so rrep accumulators stay serial
SCORE_ITEMS = [(j, c) for j in range(NJ) for c in range(4 * j + 4)]
ET_OFF = {}
_off = 0
for _j, _c in SCORE_ITEMS:
    ET_OFF[(_j, _c)] = _off
    _off += CH
ET_W = _off              # 40 * 512 fp16 = 40KB/partition


def _build():
    nc = bacc.Bacc(trn_type="TRN2")
    qt_d = nc.dram_tensor("qt", (D, L), FP16, kind="ExternalInput")
    kt_d = nc.dram_tensor("kt", (D, L), FP16, kind="ExternalInput")
    vt_d = nc.dram_tensor("vt", (D, L), FP16, kind="ExternalInput")
    # weights pre-packed on host to the SBUF image: (128, NDC*dk)
    wq_d = nc.dram_tensor("wq", (P, NDC * DK), FP16, kind="ExternalInput")
    wk_d = nc.dram_tensor("wk", (P, NDC * DK), FP16, kind="ExternalInput")
    wv_d = nc.dram_tensor("wv", (P, NDC * DV), FP16, kind="ExternalInput")
    wo_d = nc.dram_tensor("wo", (DV, D), FP16, kind="ExternalInput")
    mask_d = nc.dram_tensor("mask", (P, NJ * CH), FP16, kind="ExternalInput")
    y_d = nc.dram_tensor("y", (L, D), YDT, kind="ExternalOutput")

    with tile.TileContext(nc) as tc:
        with (
            tc.tile_pool(name="const", bufs=1) as const,
            tc.tile_pool(name="wpool", bufs=1) as wpool,
            tc.tile_pool(name="xs", bufs=6) as xs,
            tc.tile_pool(name="proj", bufs=1) as proj,
            tc.tile_pool(name="rinvp", bufs=2) as rinvp,
            tc.tile_pool(name="ev", bufs=4) as ev_pool,
            tc.tile_pool(name="ps", bufs=7, space="PSUM") as ps,
        ):
            ident = const.tile([P, P], FP16)
            make_identity(nc, ident[:])
            ones = const.tile([P, P], FP16)
            nc.vector.memset(ones[:], 1.0)
            ones2 = const.tile([P, 256], FP16)
            nc.vector.memset(ones2[:], 1.0)
            ebias = const.tile([P, 1], FP32)
            nc.vector.memset(ebias[:], EBIAS)

            kT = proj.tile([P, L], FP16, tag="kT")
            qT = proj.tile([P, L], FP16, tag="qT")
            vT = proj.tile([P, L], FP16, tag="vT")
            v_nat = proj.tile([P, L], FP16, tag="v_nat")
            oT = proj.tile([P, L], FP16, tag="oT")
            et_all = proj.tile([P, ET_W], FP16, tag="et_all")
            rinv_all = proj.tile([P, NJ * CH], FP32, tag="rinv_all")

            w_sbs = {}
            maskt = const.tile([P, NJ * CH], FP16)

            def load_w(name, wd):
                w_sb = wpool.tile([P, NDC * DK], FP16, tag=name, name=name)
                nc.scalar.dma_start(w_sb[:], wd[:])
                w_sbs[name] = w_sb

            warm = ps.tile([P, 256], FP32, tag="warm", bufs=1, name="warm")

            def project(xt_dram, wname, dst, fill=False):
                w_sb = w_sbs[wname]
                accs = [ps.tile([P, CH], FP32, tag="ps", name=f"acc{j}")
                        for j in range(NJ)]
                for dc in range(NDC):
                    xt = xs.tile([P, L], FP16, tag="xt", name="xt")
                    nc.sync.dma_start(xt[:], xt_dram[dc * P:(dc + 1) * P, :])
                    for j in range(NJ):
                        nc.tensor.matmul(
                            accs[j][:], w_sb[:, dc * P:dc * P + P],
                            xt[:, j * CH:(j + 1) * CH],
                            start=(dc == 0), stop=(dc == NDC - 1))
                    if fill:
                        # keep the PE HAM window busy while the wire streams
                        nc.tensor.matmul(warm[:], ones[:], ones2[:],
                                         start=True, stop=True)
                for j in range(NJ):
                    nc.any.tensor_copy(dst[:, j * CH:(j + 1) * CH], accs[j][:])

            # --- phase 1: k and q projections ---
            load_w("wk", wk_d)
            project(kt_d, "wk", kT[:], fill=True)
            load_w("wq", wq_d)
            nc.scalar.dma_start(maskt[:], mask_d[:])
            project(qt_d, "wq", qT[:], fill=True)

            # --- phase 2: v projection interleaved with attention scores ---
            load_w("wv", wv_d)
            wo_sb = wpool.tile([DV, D], FP16)
            nc.scalar.dma_start(wo_sb[:], wo_d[:])

            rrep = {}

            def score_item(j, c):
                st = ps.tile([P, CH], FP32, tag="ps", name="st")
                nc.tensor.matmul(st[:], kT[:, c * P:(c + 1) * P],
                                 qT[:, j * CH:(j + 1) * CH],
                                 start=True, stop=True)
                et = et_all[:, ET_OFF[(j, c)]:ET_OFF[(j, c)] + CH]
                nc.scalar.activation(et, st[:],
                                     mybir.ActivationFunctionType.Exp,
                                     bias=ebias[:], scale=SCALE)
                d = c - 4 * j
                if d >= 0:   # diagonal tile: zero out k > q
                    nc.vector.tensor_mul(et, et, maskt[:, d * CH:(d + 1) * CH])
                if c == 0:
                    rrep[j] = ps.tile([P, CH], FP32, tag="ps", name=f"rrep{j}")
                nc.tensor.matmul(rrep[j][:], ones[:], et,
                                 start=(c == 0), stop=(c == 4 * j + 3))
                if c == 4 * j + 3:
                    rinv = rinv_all[:, j * CH:(j + 1) * CH]
                    nc.vector.reciprocal_approx_fast(rinv, rrep[j][:])

            vaccs = [ps.tile([P, CH], FP32, tag="ps", name=f"vacc{j}")
                     for j in range(NJ)]
            w_sb = w_sbs["wv"]
            si = 0
            for dc in range(NDC):
                # scores first in program order: they are always ready, so the
                # in-order PE queue fills DMA-wait time with them
                nxt = (len(SCORE_ITEMS) * (dc + 1)) // NDC
                while si < nxt:
                    score_item(*SCORE_ITEMS[si])
                    si += 1
                xt = xs.tile([P, L], FP16, tag="xt", name="xt")
                nc.sync.dma_start(xt[:], vt_d[dc * P:(dc + 1) * P, :])
                for j in range(NJ):
                    nc.tensor.matmul(
                        vaccs[j][:], w_sb[:, dc * P:dc * P + P],
                        xt[:, j * CH:(j + 1) * CH],
                        start=(dc == 0), stop=(dc == NDC - 1))
            while si < len(SCORE_ITEMS):
                score_item(*SCORE_ITEMS[si])
                si += 1
            for j in range(NJ):
                nc.any.tensor_copy(vT[:, j * CH:(j + 1) * CH], vaccs[j][:])

            # --- phase 3: v transposes, PV, normalize, Y, all pipelined ---
            def transposes(j):
                for c in range(4 * j, 4 * j + 4):
                    tp = ps.tile([P, P], FP16, tag="ps", name="tp")
                    nc.tensor.transpose(tp[:], vT[:, c * P:(c + 1) * P], ident[:])
                    nc.any.tensor_copy(v_nat[:, c * P:(c + 1) * P], tp[:])

            def ot_chunk(j):
                ot = ps.tile([P, CH], FP32, tag="ps", name="ot")
                for c in range(4 * j + 4):
                    nc.tensor.matmul(ot[:], v_nat[:, c * P:(c + 1) * P],
                                     et_all[:, ET_OFF[(j, c)]:ET_OFF[(j, c)] + CH],
                                     start=(c == 0), stop=(c == 4 * j + 3))
                nc.vector.tensor_mul(oT[:, j * CH:(j + 1) * CH], ot[:],
                                     rinv_all[:, j * CH:(j + 1) * CH])

            def y_chunk(j, split=False):
                for t in range(CH // P):
                    lq0 = j * CH + t * P
                    yev = ev_pool.tile([P, D], YDT, tag="yev", name="yev")
                    for dch in range(D // CH):
                        yps = ps.tile([P, CH], FP32, tag="ps", name="yps")
                        nc.tensor.matmul(yps[:], oT[:, lq0:lq0 + P],
                                         wo_sb[:, dch * CH:(dch + 1) * CH],
                                         start=True, stop=True)
                        dst = yev[:, dch * CH:(dch + 1) * CH]
                        if dch % 2 == 0:
                            nc.vector.tensor_copy(dst, yps[:])
                        else:
                            nc.scalar.copy(dst, yps[:])
                        if split:
                            nc.sync.dma_start(
                                y_d[lq0:lq0 + P, dch * CH:(dch + 1) * CH], dst)
                    if not split:
                        nc.sync.dma_start(y_d[lq0:lq0 + P, :], yev[:])

            # software pipeline: Y(j-1) hides behind OT(j)/transposes(j+1)
            transposes(0)
            ot_chunk(0)
            for j in range(1, NJ):
                transposes(j)
                ot_chunk(j)
                y_chunk(j - 1)
            y_chunk(NJ - 1)
    nc.compile()
    return nc


_NC = None


def _get_nc():
    global _NC
    if _NC is None:
        _NC = _build()
    return _NC


def _pack_w(w):
    """(D, dk) fp32 -> SBUF image (128, NDC*dk): out[p, dc*dk+m] = w[dc*128+p, m]"""
    return np.ascontiguousarray(
        w.reshape(NDC, P, -1).transpose(1, 0, 2).reshape(P, -1)).astype(np.float16)


def _make_in_maps(Q, K, V, Wq, Wk, Wv, Wo):
    f16 = np.float16
    # fold GQA group sum into Wq: head = g*KV_HEADS + h
    Wq_eff = np.asarray(Wq, np.float32).reshape(D, GROUPS, KV_HEADS, DK).sum(axis=1)
    mask = np.zeros((P, NJ * CH), f16)
    for d in range(4):
        p = np.arange(P)[:, None]
        x = np.arange(CH)[None, :]
        mask[:, d * CH:(d + 1) * CH] = (128 * d + p <= x).astype(f16)
    acts = {}
    for b in range(B):
        acts[b] = {
            "qt": np.ascontiguousarray(np.asarray(Q[b], np.float32).T).astype(f16),
            "kt": np.ascontiguousarray(np.asarray(K[b], np.float32).T).astype(f16),
            "vt": np.ascontiguousarray(np.asarray(V[b], np.float32).T).astype(f16),
        }
    Wk32, Wv32 = np.asarray(Wk, np.float32), np.asarray(Wv, np.float32)
    Wo32 = np.asarray(Wo, np.float32)
    in_maps = []
    for c in range(8):
        b, h = divmod(c, KV_HEADS)
        in_maps.append({
            **acts[b],
            "wq": _pack_w(Wq_eff[:, h, :]),
            "wk": _pack_w(Wk32[:, h * DK:(h + 1) * DK]),
            "wv": _pack_w(Wv32[:, h * DV:(h + 1) * DV]),
            "wo": Wo32[h * DV:(h + 1) * DV, :].astype(f16),
            "mask": mask,
        })
    return in_maps


def _gather(results):
    Y = np.zeros((B, L, D), np.float32)
    for c in range(8):
        Y[c // KV_HEADS] += results[c]["y"].astype(np.float32)
    return Y


def kernel(Q, K, V, Wq, Wk, Wv, Wo):
    nc = _get_nc()
    in_maps = _make_in_maps(Q, K, V, Wq, Wk, Wv, Wo)
    res = bass_utils.run_bass_kernel_spmd(nc, in_maps, core_ids=list(range(8)))
    return _gather(res.results)


def _install_ntff_hook():
    """The agent image's antenv lacks axon_hooks; synthesize it so
    trace=True can reach the NTFF profiler in libaxon_pjrt.so."""
    import types
    import antenv
    if hasattr(antenv, "axon_hooks"):
        return
    mod = types.ModuleType("antenv.axon_hooks")
    _h = [None]
    mod.set_axon_ntff_profile_hook = lambda h: _h.__setitem__(0, h)
    mod.get_axon_ntff_profile_hook = lambda: _h[0]
    sys.modules["antenv.axon_hooks"] = mod
    antenv.axon_hooks = mod
    from trn_agent_boot.trn_boot import _ntff_profile_via_ctypes
    mod.set_axon_ntff_profile_hook(_ntff_profile_via_ctypes("/opt/axon/libaxon_pjrt.so"))


def kernel_traced(Q, K, V, Wq, Wk, Wv, Wo):
    """Like kernel() but profiles; returns (output, BassKernelResults)."""
    _install_ntff_hook()
    nc = _get_nc()
    in_maps = _make_in_maps(Q, K, V, Wq, Wk, Wv, Wo)
    res = bass_utils.run_bass_kernel_spmd(nc, in_maps, core_ids=list(range(8)),
                                          trace=True)
    return _gather(res.results), res



# revision 3
# speedup vs baseline: 1.0190x; 1.0190x over previous
"""GQA kernel for Trainium2, sharded over 8 NeuronCores.

Sharding: data-parallel over batch (2) x tensor-parallel over kv_heads (4).
Core c = b*4 + h computes the full attention output partial
    Y_bh = softmax(causal((Q_b @ Wq_eff_h) @ (K_b @ Wk_h)^T / sqrt(dk))) @ (V_b @ Wv_h) @ Wo_h
and the host sums the 4 head partials per batch (the "all-reduce after Wo").

The GQA group-sum-before-softmax quirk folds into the weights:
    scores_h = sum_g (Q Wq_{g,h}) (K Wk_h)^T = (Q [sum_g Wq_{g,h}]) (K Wk_h)^T
so Wq_eff_h = sum_g Wq[:, (g*KV+h)*dk : ...] and each core runs standard attention.

Device schedule (fully pipelined, wire and PE both ~85% of the span):
  All activations are streamed COLUMN-major (per 512-seq slab, host-packed
  into the SBUF image so each 512KB sub-DMA is contiguous): a slab's
  projection completes right after its 2MB lands, so downstream work starts
  ~4.6us after each slab instead of after the whole 8MB stream.
    k0 q0 k1 q1 k2 q2 k3 q3 | v0 v1 v2 v3     (sync-queue order)
  Scores for query chunk j (S^T tiles -> exp -> causal mask -> fp16 row-sum
  accumulation on Vector) are interleaved one-at-a-time into the NEXT
  slab's projection matmuls, so the scalar-engine exp latency (577ns/tile)
  never heads-of-line-blocks the in-order PE queue.  Row sums use vector
  adds of the exp'd tiles plus ONE ones-matmul per chunk (saves 36 PE
  matmuls vs. per-tile ones-matmuls).
  After v key-chunk m lands: v-proj(m) -> transposes -> PV updates for all
  j>=m (ot[j] PSUM banks accumulate across m) -> normalize chunk m ->
  Y(m) = O_m @ Wo evicted and DMA'd out on the gpsimd queue.  Output
  stores are therefore spread over the whole v phase instead of bursting
  at the end; the tail after the last v byte is ~6us of PE.

PSUM: 8 banks = acc ring(2) + work ring(2: score/transpose/rowsum/Y) +
ot[0..3] (4 persistent PV accumulators).
"""
import sys
sys.path.insert(0, '/opt/trn_rl_repo')
import math
import numpy as np

import concourse.bass as bass
import concourse.mybir as mybir
import concourse.tile as tile
from concourse import bacc
from concourse import bass_utils
from concourse.masks import make_identity

FP32 = mybir.dt.float32
FP16 = mybir.dt.float16

B, L, D = 2, 2048, 2048
Q_HEADS, KV_HEADS, DK, DV = 16, 4, 128, 128
GROUPS = Q_HEADS // KV_HEADS
P = 128
CH = 512                 # seq slab width (queries and keys)
NJ = L // CH             # 4 slabs
NDC = D // P             # 16 contraction chunks
NSUB = 4                 # sub-DMAs per slab (4 dc each, 512KB)
SCALE = 1.0 / math.sqrt(DK)
EBIAS = -8.0 * math.log(2.0)   # exp output scaled by 2^-8; cancels in softmax
YDT = FP16               # partial-output dtype (host accumulates in fp32)

# et tile offsets, (j, c) j-major causal
ET_OFF = {}
_off = 0
for _j in range(NJ):
    for _c in range(4 * _j + 4):
        ET_OFF[(_j, _c)] = _off
        _off += CH
ET_W = _off              # 40 * 512 fp16 = 40KB/partition


def _build():
    nc = bacc.Bacc(trn_type="TRN2")
    # activations host-packed: row (j*4+s)*128+p, col dcs*512+c holds
    # X[j*512+c, (s*4+dcs)*128+p]; each [128,2048] row-block is one
    # contiguous 512KB sub-slab covering d-chunks [4s, 4s+4) of seq slab j.
    qx_d = nc.dram_tensor("qx", (L, D), FP16, kind="ExternalInput")
    kx_d = nc.dram_tensor("kx", (L, D), FP16, kind="ExternalInput")
    vx_d = nc.dram_tensor("vx", (L, D), FP16, kind="ExternalInput")
    # weights pre-packed on host to the SBUF image: (128, NDC*dk)
    wq_d = nc.dram_tensor("wq", (P, NDC * DK), FP16, kind="ExternalInput")
    wk_d = nc.dram_tensor("wk", (P, NDC * DK), FP16, kind="ExternalInput")
    wv_d = nc.dram_tensor("wv", (P, NDC * DV), FP16, kind="ExternalInput")
    wo_d = nc.dram_tensor("wo", (DV, D), FP16, kind="ExternalInput")
    mask_d = nc.dram_tensor("mask", (P, NJ * CH), FP16, kind="ExternalInput")
    y_d = nc.dram_tensor("y", (L, D), YDT, kind="ExternalOutput")

    with tile.TileContext(nc) as tc:
        with (
            tc.tile_pool(name="const", bufs=1) as const,
            tc.tile_pool(name="wpool", bufs=1) as wpool,
            tc.tile_pool(name="xs", bufs=12) as xs,
            tc.tile_pool(name="proj", bufs=1) as proj,
            tc.tile_pool(name="ev", bufs=3) as ev_pool,
            tc.tile_pool(name="ps", bufs=2, space="PSUM") as ps,
        ):
            ident = const.tile([P, P], FP16)
            make_identity(nc, ident[:])
            ones = const.tile([P, P], FP16)
            nc.vector.memset(ones[:], 1.0)
            ebias = const.tile([P, 1], FP32)
            nc.vector.memset(ebias[:], EBIAS)
            maskt = const.tile([P, NJ * CH], FP16)

            kT = proj.tile([P, L], FP16, tag="kT")
            qT = proj.tile([P, L], FP16, tag="qT")
            v_nat = proj.tile([P, L], FP16, tag="v_nat")
            oT = proj.tile([P, L], FP16, tag="oT")
            et_all = proj.tile([P, ET_W], FP16, tag="et_all")
            ssum = proj.tile([P, NJ * CH], FP16, tag="ssum")
            rinv_all = proj.tile([P, NJ * CH], FP32, tag="rinv_all")

            wk_sb = wpool.tile([P, NDC * DK], FP16, tag="wk")
            wq_sb = wpool.tile([P, NDC * DK], FP16, tag="wq")
            wv_sb = wpool.tile([P, NDC * DV], FP16, tag="wv")
            wo_sb = wpool.tile([DV, D], FP16, tag="wo")

            # ---- helpers ----------------------------------------------
            def dma_w_subs(w_sb, w_d):
                for s in range(NSUB):
                    nc.sync.dma_start(w_sb[:, s * 4 * P:(s + 1) * 4 * P],
                                      w_d[:, s * 4 * P:(s + 1) * 4 * P])

            score_pend = []   # (j, c) items ready to emit
            si = [0]

            def score_item(j, c):
                st = ps.tile([P, CH], FP32, tag="work", name="st")
                nc.tensor.matmul(st[:], kT[:, c * P:(c + 1) * P],
                                 qT[:, j * CH:(j + 1) * CH],
                                 start=True, stop=True)
                et = et_all[:, ET_OFF[(j, c)]:ET_OFF[(j, c)] + CH]
                nc.scalar.activation(et, st[:],
                                     mybir.ActivationFunctionType.Exp,
                                     bias=ebias[:], scale=SCALE)
                d = c - 4 * j
                if d >= 0:   # diagonal tile: zero out k > q
                    nc.vector.tensor_mul(et, et, maskt[:, d * CH:(d + 1) * CH])
                ss = ssum[:, j * CH:(j + 1) * CH]
                if c == 1:
                    e0 = et_all[:, ET_OFF[(j, 0)]:ET_OFF[(j, 0)] + CH]
                    nc.vector.tensor_add(ss, e0, et)
                elif c > 1:
                    nc.vector.tensor_add(ss, ss, et)

            def emit_scores(n):
                while n > 0 and si[0] < len(score_pend):
                    score_item(*score_pend[si[0]])
                    si[0] += 1
                    n -= 1

            def emit_rrep(j):
                rrep = ps.tile([P, CH], FP32, tag="work", name="rrep")
                nc.tensor.matmul(rrep[:], ones[:],
                                 ssum[:, j * CH:(j + 1) * CH],
                                 start=True, stop=True)
                nc.vector.reciprocal_approx_fast(
                    rinv_all[:, j * CH:(j + 1) * CH], rrep[:])

            def proj_slab(x_d, w_sb, j, acc_name, emits):
                acc = ps.tile([P, CH], FP32, tag="acc", name=acc_name)
                for s in range(NSUB):
                    xt = xs.tile([P, NSUB * CH], FP16, tag="xt", name="xt")
                    r0 = (j * NSUB + s) * P
                    nc.sync.dma_start(xt[:], x_d[r0:r0 + P, :])
                    for dcs in range(4):
                        dc = s * 4 + dcs
                        nc.tensor.matmul(
                            acc[:], w_sb[:, dc * P:(dc + 1) * P],
                            xt[:, dcs * CH:(dcs + 1) * CH],
                            start=(dc == 0), stop=(dc == NDC - 1))
                    emit_scores(emits[s])
                return acc

            # ---- k/q interleaved streaming phase ----------------------
            dma_w_subs(wk_sb, wk_d)
            SC_K = {0: (0, 0, 0, 0), 1: (0, 1, 1, 1), 2: (0, 1, 1, 1),
                    3: (1, 2, 1, 2)}
            SC_Q = {0: (0, 0, 0, 0), 1: (1, 0, 0, 0), 2: (1, 1, 1, 2),
                    3: (1, 2, 1, 2)}
            for j in range(NJ):
                kacc = proj_slab(kx_d, wk_sb, j, "kacc", SC_K[j])
                nc.any.tensor_copy(kT[:, j * CH:(j + 1) * CH], kacc[:])
                if j == 0:
                    dma_w_subs(wq_sb, wq_d)
                qacc = proj_slab(qx_d, wq_sb, j, "qacc", SC_Q[j])
                nc.any.tensor_copy(qT[:, j * CH:(j + 1) * CH], qacc[:])
                if j == 0:
                    nc.sync.dma_start(maskt[:], mask_d[:])
                for c in range(4 * j + 4):
                    score_pend.append((j, c))
                if j == 1:
                    emit_scores(1)
                    emit_rrep(0)
                elif j == 2:
                    emit_scores(1)
                    emit_rrep(1)
                elif j == 3:
                    emit_scores(2)

            # ---- v phase: per key chunk m: vproj, PV, normalize, Y ----
            dma_w_subs(wv_sb, wv_d)
            ot = [ps.tile([P, CH], FP32, tag="ot", bufs=4, name=f"ot{j}")
                  for j in range(NJ)]
            SC_V = {0: (2, 2, 2, 2), 1: (2, 2, 2, 2), 2: (0, 0, 0, 0),
                    3: (0, 0, 0, 0)}
            for m in range(NJ):
                vacc = proj_slab(vx_d, wv_sb, m, "vacc", SC_V[m])
                if m == 0:
                    nc.sync.dma_start(wo_sb[:], wo_d[:])
                vTc = proj.tile([P, CH], FP16, tag="vTc", bufs=2, name="vTc")
                nc.any.tensor_copy(vTc[:], vacc[:])
                emit_scores(4)
                if m == 0:
                    emit_rrep(2)
                for t in range(4):
                    tp = ps.tile([P, P], FP16, tag="work", name="tp")
                    nc.tensor.transpose(tp[:], vTc[:, t * P:(t + 1) * P],
                                        ident[:])
                    nc.any.tensor_copy(
                        v_nat[:, (4 * m + t) * P:(4 * m + t + 1) * P], tp[:])
                emit_scores(4)
                if m == 1:
                    emit_rrep(3)
                # PV for j == m first so normalize can overlap later PV
                for j in range(m, NJ):
                    for t in range(4):
                        c = 4 * m + t
                        nc.tensor.matmul(
                            ot[j][:], v_nat[:, c * P:(c + 1) * P],
                            et_all[:, ET_OFF[(j, c)]:ET_OFF[(j, c)] + CH],
                            start=(c == 0), stop=(c == 4 * j + 3))
                    if j == m:
                        # piecewise normalize so Y pieces start ASAP
                        for t in range(4):
                            lq = m * CH + t * P
                            nc.vector.tensor_mul(
                                oT[:, lq:lq + P], ot[m][:, t * P:(t + 1) * P],
                                rinv_all[:, lq:lq + P])
                # Y(m) = O_m @ Wo, evicted + stored per 128-row piece
                for t in range(4):
                    lq0 = m * CH + t * P
                    yev = ev_pool.tile([P, D], YDT, tag="yev", name="yev")
                    for dch in range(D // CH):
                        yps = ps.tile([P, CH], FP32, tag="work", name="yps")
                        nc.tensor.matmul(yps[:], oT[:, lq0:lq0 + P],
                                         wo_sb[:, dch * CH:(dch + 1) * CH],
                                         start=True, stop=True)
                        dst = yev[:, dch * CH:(dch + 1) * CH]
                        if dch % 2 == 0:
                            nc.vector.tensor_copy(dst, yps[:])
                        else:
                            nc.scalar.copy(dst, yps[:])
                    nc.gpsimd.dma_start(y_d[lq0:lq0 + P, :], yev[:])
            assert si[0] == len(score_pend) == len(ET_OFF)
    nc.compile()
    return nc


_NC = None


def _get_nc():
    global _NC
    if _NC is None:
        _NC = _build()
    return _NC


def _pack_w(w):
    """(D, dk) fp32 -> SBUF image (128, NDC*dk): out[p, dc*dk+m] = w[dc*128+p, m]"""
    return np.ascontiguousarray(
        w.reshape(-1, P, w.shape[-1]).transpose(1, 0, 2).reshape(P, -1)).astype(np.float16)


def _pack_act(x):
    """(L, D) fp32 -> packed fp16 (L, D): row (j*4+s)*128+p, col dcs*512+c
    holds x[j*512+c, (s*4+dcs)*128+p]."""
    xt = np.ascontiguousarray(np.asarray(x, np.float32).T)   # (D, L)
    a = xt.reshape(NSUB, 4, P, NJ, CH)        # [s, dcs, p, j, c]
    a = a.transpose(3, 0, 2, 1, 4)            # [j, s, p, dcs, c]
    return np.ascontiguousarray(a.reshape(L, D)).astype(np.float16)


def _make_in_maps(Q, K, V, Wq, Wk, Wv, Wo):
    f16 = np.float16
    # fold GQA group sum into Wq: head = g*KV_HEADS + h
    Wq_eff = np.asarray(Wq, np.float32).reshape(D, GROUPS, KV_HEADS, DK).sum(axis=1)
    mask = np.zeros((P, NJ * CH), f16)
    for d in range(4):
        p = np.arange(P)[:, None]
        x = np.arange(CH)[None, :]
        mask[:, d * CH:(d + 1) * CH] = (128 * d + p <= x).astype(f16)
    acts = {}
    for b in range(B):
        acts[b] = {
            "qx": _pack_act(Q[b]),
            "kx": _pack_act(K[b]),
            "vx": _pack_act(V[b]),
        }
    Wk32, Wv32 = np.asarray(Wk, np.float32), np.asarray(Wv, np.float32)
    Wo32 = np.asarray(Wo, np.float32)
    in_maps = []
    for c in range(8):
        b, h = divmod(c, KV_HEADS)
        in_maps.append({
            **acts[b],
            "wq": _pack_w(Wq_eff[:, h, :]),
            "wk": _pack_w(Wk32[:, h * DK:(h + 1) * DK]),
            "wv": _pack_w(Wv32[:, h * DV:(h + 1) * DV]),
            "wo": Wo32[h * DV:(h + 1) * DV, :].astype(f16),
            "mask": mask,
        })
    return in_maps


def _gather(results):
    Y = np.zeros((B, L, D), np.float32)
    for c in range(8):
        Y[c // KV_HEADS] += results[c]["y"].astype(np.float32)
    return Y


def kernel(Q, K, V, Wq, Wk, Wv, Wo):
    nc = _get_nc()
    in_maps = _make_in_maps(Q, K, V, Wq, Wk, Wv, Wo)
    res = bass_utils.run_bass_kernel_spmd(nc, in_maps, core_ids=list(range(8)))
    return _gather(res.results)


def _install_ntff_hook():
    """The agent image's antenv lacks axon_hooks; synthesize it so
    trace=True can reach the NTFF profiler in libaxon_pjrt.so."""
    import types
    import antenv
    if hasattr(antenv, "axon_hooks"):
        return
    mod = types.ModuleType("antenv.axon_hooks")
    _h = [None]
    mod.set_axon_ntff_profile_hook = lambda h: _h.__setitem__(0, h)
    mod.get_axon_ntff_profile_hook = lambda: _h[0]
    sys.modules["antenv.axon_hooks"] = mod
    antenv.axon_hooks = mod
    from trn_agent_boot.trn_boot import _ntff_profile_via_ctypes
    mod.set_axon_ntff_profile_hook(_ntff_profile_via_ctypes("/opt/axon/libaxon_pjrt.so"))


def kernel_traced(Q, K, V, Wq, Wk, Wv, Wo):
    """Like kernel() but profiles; returns (output, BassKernelResults)."""
    _install_ntff_hook()
    nc = _get_nc()
    in_maps = _make_in_maps(Q, K, V, Wq, Wk, Wv, Wo)
    res = bass_utils.run_bass_kernel_spmd(nc, in_maps, core_ids=list(range(8)),
                                          trace=True)
    return _gather(res.results), res


# revision 4
# speedup vs baseline: 1.0391x; 1.0198x over previous
"""GQA kernel for Trainium2, sharded over 8 NeuronCores.

Sharding: data-parallel over batch (2) x tensor-parallel over kv_heads (4).
Core c = b*4 + h computes the full attention output partial
    Y_bh = softmax(causal((Q_b @ Wq_eff_h) @ (K_b @ Wk_h)^T / sqrt(dk))) @ (V_b @ Wv_h) @ Wo_h
and the host sums the 4 head partials per batch (the "all-reduce after Wo").

The GQA group-sum-before-softmax quirk folds into the weights:
    scores_h = sum_g (Q Wq_{g,h}) (K Wk_h)^T = (Q [sum_g Wq_{g,h}]) (K Wk_h)^T
so Wq_eff_h = sum_g Wq[:, (g*KV+h)*dk : ...] and each core runs standard attention.

Device schedule (fully pipelined; wire ~83us and PE ~77us both near-saturated):
  All activations stream COLUMN-major (per 512-seq slab, host-packed into the
  SBUF image so each 512KB sub-DMA is one contiguous row-block): a slab's
  projection completes right after its 2MB lands, so downstream work starts
  ~5us after each slab instead of after the whole 8MB stream.
  K and Q slabs stream pairwise (k0 q0 k1 q1 ...) with their projection
  matmuls interleaved one-by-one: consecutive matmuls alternate PSUM banks,
  avoiding the ~2x same-bank back-to-back accumulation stall.
  Scores for chunk j (S^T tiles -> exp -> causal mask -> fp16 row-sum adds on
  Vector + ONE ones-matmul per chunk) are drip-fed into the next stage's
  matmul stream so the scalar exp latency never blocks the in-order PE queue.
  The v phase is software-pipelined: v-proj(m) matmuls interleave with the
  compute of chunk m-1 (transposes, PV j-rotated across the 4 persistent ot
  banks, normalize, Y = O @ Wo, output stores on the gpsimd queue). Output
  stores spread over the whole v phase; the post-stream tail is ~8us.

PSUM: 8 banks = acc ring(2) + work ring(2: score/transpose/rowsum/Y) +
ot[0..3] (4 persistent PV accumulators).
"""
import sys
sys.path.insert(0, '/opt/trn_rl_repo')
import math
import numpy as np

import concourse.bass as bass
import concourse.mybir as mybir
import concourse.tile as tile
from concourse import bacc
from concourse import bass_utils
from concourse.masks import make_identity

FP32 = mybir.dt.float32
FP16 = mybir.dt.float16

B, L, D = 2, 2048, 2048
Q_HEADS, KV_HEADS, DK, DV = 16, 4, 128, 128
GROUPS = Q_HEADS // KV_HEADS
P = 128
CH = 512                 # seq slab width (queries and keys)
NJ = L // CH             # 4 slabs
NDC = D // P             # 16 contraction chunks
NSUB = 4                 # sub-DMAs per slab (4 dc each, 512KB)
SCALE = 1.0 / math.sqrt(DK)
EBIAS = -8.0 * math.log(2.0)   # exp output scaled by 2^-8; cancels in softmax
YDT = FP16               # partial-output dtype (host accumulates in fp32)

# et tile offsets, (j, c) j-major causal
ET_OFF = {}
_off = 0
for _j in range(NJ):
    for _c in range(4 * _j + 4):
        ET_OFF[(_j, _c)] = _off
        _off += CH
ET_W = _off              # 40 * 512 fp16 = 40KB/partition


def _build():
    nc = bacc.Bacc(trn_type="TRN2")
    # activations host-packed: row (j*4+s)*128+p, col dcs*512+c holds
    # X[j*512+c, (s*4+dcs)*128+p]; each [128,2048] row-block is one
    # contiguous 512KB sub-slab covering d-chunks [4s, 4s+4) of seq slab j.
    qx_d = nc.dram_tensor("qx", (L, D), FP16, kind="ExternalInput")
    kx_d = nc.dram_tensor("kx", (L, D), FP16, kind="ExternalInput")
    vx_d = nc.dram_tensor("vx", (L, D), FP16, kind="ExternalInput")
    # weights pre-packed on host to the SBUF image: (128, NDC*dk)
    wq_d = nc.dram_tensor("wq", (P, NDC * DK), FP16, kind="ExternalInput")
    wk_d = nc.dram_tensor("wk", (P, NDC * DK), FP16, kind="ExternalInput")
    wv_d = nc.dram_tensor("wv", (P, NDC * DV), FP16, kind="ExternalInput")
    wo_d = nc.dram_tensor("wo", (DV, D), FP16, kind="ExternalInput")
    y_d = nc.dram_tensor("y", (L, D), YDT, kind="ExternalOutput")

    with tile.TileContext(nc) as tc:
        with (
            tc.tile_pool(name="const", bufs=1) as const,
            tc.tile_pool(name="wpool", bufs=1) as wpool,
            tc.tile_pool(name="xs", bufs=12) as xs,
            tc.tile_pool(name="proj", bufs=1) as proj,
            tc.tile_pool(name="ev", bufs=3) as ev_pool,
            tc.tile_pool(name="ps", bufs=2, space="PSUM") as ps,
        ):
            ident = const.tile([P, P], FP16)
            make_identity(nc, ident[:])
            ones = const.tile([P, P], FP16)
            nc.vector.memset(ones[:], 1.0)
            ebias = const.tile([P, 1], FP32)
            nc.vector.memset(ebias[:], EBIAS)
            # causal mask, built on-device: maskt[p, d*CH+x] = (128d+p <= x)
            maskt = const.tile([P, NJ * CH], FP16)
            nc.gpsimd.memset(maskt[:], 1.0)
            for dd in range(4):
                nc.gpsimd.affine_select(
                    out=maskt[:, dd * CH:(dd + 1) * CH],
                    in_=maskt[:, dd * CH:(dd + 1) * CH],
                    compare_op=mybir.AluOpType.is_ge,
                    fill=0.0, base=-128 * dd,
                    pattern=[[1, CH]], channel_multiplier=-1)

            kT = proj.tile([P, L], FP16, tag="kT")
            qT = proj.tile([P, L], FP16, tag="qT")
            v_nat = proj.tile([P, L], FP16, tag="v_nat")
            oT = proj.tile([P, L], FP16, tag="oT")
            et_all = proj.tile([P, ET_W], FP16, tag="et_all")
            ssum = proj.tile([P, NJ * CH], FP16, tag="ssum")
            rinv_all = proj.tile([P, NJ * CH], FP32, tag="rinv_all")

            wk_sb = wpool.tile([P, NDC * DK], FP16, tag="wk")
            wq_sb = wpool.tile([P, NDC * DK], FP16, tag="wq")
            wv_sb = wpool.tile([P, NDC * DV], FP16, tag="wv")
            wo_sb = wpool.tile([DV, D], FP16, tag="wo")

            # ---- score items, drip-fed between projection matmuls ------
            score_pend = []
            si = [0]

            def score_item(j, c):
                st = ps.tile([P, CH], FP32, tag="work", name="st")
                nc.tensor.matmul(st[:], kT[:, c * P:(c + 1) * P],
                                 qT[:, j * CH:(j + 1) * CH],
                                 start=True, stop=True)
                et = et_all[:, ET_OFF[(j, c)]:ET_OFF[(j, c)] + CH]
                nc.scalar.activation(et, st[:],
                                     mybir.ActivationFunctionType.Exp,
                                     bias=ebias[:], scale=SCALE)
                d = c - 4 * j
                if d >= 0:   # diagonal tile: zero out k > q
                    nc.vector.tensor_mul(et, et, maskt[:, d * CH:(d + 1) * CH])
                ss = ssum[:, j * CH:(j + 1) * CH]
                if c == 1:
                    e0 = et_all[:, ET_OFF[(j, 0)]:ET_OFF[(j, 0)] + CH]
                    nc.vector.tensor_add(ss, e0, et)
                elif c > 1:
                    nc.vector.tensor_add(ss, ss, et)

            def emit_scores(n):
                while n > 0 and si[0] < len(score_pend):
                    score_item(*score_pend[si[0]])
                    si[0] += 1
                    n -= 1

            def emit_rrep(j):
                rrep = ps.tile([P, CH], FP32, tag="work", name="rrep")
                nc.tensor.matmul(rrep[:], ones[:],
                                 ssum[:, j * CH:(j + 1) * CH],
                                 start=True, stop=True)
                nc.vector.reciprocal_approx_fast(
                    rinv_all[:, j * CH:(j + 1) * CH], rrep[:])

            # ---- k/q pairwise-interleaved streaming phase --------------
            nc.scalar.dma_start(wk_sb[:], wk_d[:])
            nc.scalar.dma_start(wq_sb[:], wq_d[:])
            EMIT_KQ = {0: (0, 0, 0, 0), 1: (1, 1, 1, 1), 2: (2, 2, 2, 2),
                       3: (3, 3, 3, 3)}
            for j in range(NJ):
                kacc = ps.tile([P, CH], FP32, tag="acc", name="kacc")
                qacc = ps.tile([P, CH], FP32, tag="acc", name="qacc")
                for s in range(NSUB):
                    r0 = (j * NSUB + s) * P
                    kxt = xs.tile([P, NSUB * CH], FP16, tag="xt", name="kxt")
                    nc.sync.dma_start(kxt[:], kx_d[r0:r0 + P, :])
                    qxt = xs.tile([P, NSUB * CH], FP16, tag="xt", name="qxt")
                    nc.sync.dma_start(qxt[:], qx_d[r0:r0 + P, :])
                    for dcs in range(4):
                        dc = s * 4 + dcs
                        for acc, w_sb, xt in ((kacc, wk_sb, kxt),
                                              (qacc, wq_sb, qxt)):
                            nc.tensor.matmul(
                                acc[:], w_sb[:, dc * P:(dc + 1) * P],
                                xt[:, dcs * CH:(dcs + 1) * CH],
                                start=(dc == 0), stop=(dc == NDC - 1))
                    emit_scores(EMIT_KQ[j][s])
                nc.vector.tensor_copy(kT[:, j * CH:(j + 1) * CH], kacc[:])
                nc.scalar.copy(qT[:, j * CH:(j + 1) * CH], qacc[:])
                for c in range(4 * j + 4):
                    score_pend.append((j, c))
                if j == 2:
                    emit_rrep(0)
                elif j == 3:
                    emit_rrep(1)

            # ---- v phase, software-pipelined ---------------------------
            # compute(m-1) items (PE/vector/scalar ops) interleave with the
            # v-proj(m) matmul stream; ot[j] banks accumulate PV across m.
            nc.sync.dma_start(wv_sb[:], wv_d[:])
            ot = [ps.tile([P, CH], FP32, tag="ot", bufs=4, name=f"ot{j}")
                  for j in range(NJ)]
            comp_q = []
            ci = [0]

            def pop_comp(n):
                while n > 0 and ci[0] < len(comp_q):
                    comp_q[ci[0]]()
                    ci[0] += 1
                    n -= 1

            def make_pv(j, c):
                def f():
                    nc.tensor.matmul(
                        ot[j][:], v_nat[:, c * P:(c + 1) * P],
                        et_all[:, ET_OFF[(j, c)]:ET_OFF[(j, c)] + CH],
                        start=(c == 0), stop=(c == 4 * j + 3))
                return f

            def make_tp(m, t, vTc):
                def f():
                    tp = ps.tile([P, P], FP16, tag="work", name="tp")
                    nc.tensor.transpose(tp[:], vTc[:, t * P:(t + 1) * P],
                                        ident[:])
                    c = 4 * m + t
                    nc.vector.tensor_copy(
                        v_nat[:, c * P:(c + 1) * P], tp[:])
                return f

            def make_norm(m, t):
                def f():
                    lq = m * CH + t * P
                    nc.vector.tensor_mul(
                        oT[:, lq:lq + P], ot[m][:, t * P:(t + 1) * P],
                        rinv_all[:, lq:lq + P])
                return f

            def make_y(m, t):
                def f():
                    lq0 = m * CH + t * P
                    yev = ev_pool.tile([P, D], YDT, tag="yev", name="yev")
                    for dch in range(D // CH):
                        yps = ps.tile([P, CH], FP32, tag="work", name="yps")
                        nc.tensor.matmul(yps[:], oT[:, lq0:lq0 + P],
                                         wo_sb[:, dch * CH:(dch + 1) * CH],
                                         start=True, stop=True)
                        dst = yev[:, dch * CH:(dch + 1) * CH]
                        if dch % 2 == 0:
                            nc.vector.tensor_copy(dst, yps[:])
                        else:
                            nc.scalar.copy(dst, yps[:])
                    nc.gpsimd.dma_start(y_d[lq0:lq0 + P, :], yev[:])
                return f

            for m in range(NJ):
                vacc = ps.tile([P, CH], FP32, tag="acc", name="vacc")
                for s in range(NSUB):
                    r0 = (m * NSUB + s) * P
                    vxt = xs.tile([P, NSUB * CH], FP16, tag="xt", name="vxt")
                    nc.sync.dma_start(vxt[:], vx_d[r0:r0 + P, :])
                    if m == 0 and s == 0:
                        nc.sync.dma_start(wo_sb[:], wo_d[:])
                    for dcs in range(4):
                        dc = s * 4 + dcs
                        nc.tensor.matmul(
                            vacc[:], wv_sb[:, dc * P:(dc + 1) * P],
                            vxt[:, dcs * CH:(dcs + 1) * CH],
                            start=(dc == 0), stop=(dc == NDC - 1))
                        if m == 0:
                            emit_scores(1)
                        else:
                            pop_comp(2)
                # drain previous chunk's compute before queueing this one
                pop_comp(len(comp_q))
                if m == 0:
                    emit_rrep(2)
                elif m == 1:
                    emit_rrep(3)
                vTc = proj.tile([P, CH], FP16, tag="vTc", bufs=2, name="vTc")
                nc.vector.tensor_copy(vTc[:], vacc[:])
                for t in range(4):
                    comp_q.append(make_tp(m, t, vTc))
                for t in range(4):
                    for j in range(m, NJ):
                        comp_q.append(make_pv(j, 4 * m + t))
                    if t == 3:
                        for tt in range(4):
                            comp_q.append(make_norm(m, tt))
                for t in range(4):
                    comp_q.append(make_y(m, t))
            pop_comp(len(comp_q))
            assert si[0] == len(score_pend) == len(ET_OFF)
    nc.compile()
    return nc


_NC = None


def _get_nc():
    global _NC
    if _NC is None:
        _NC = _build()
    return _NC


def _pack_w(w):
    """(D, dk) fp32 -> SBUF image (128, NDC*dk): out[p, dc*dk+m] = w[dc*128+p, m]"""
    return np.ascontiguousarray(
        w.reshape(-1, P, w.shape[-1]).transpose(1, 0, 2).reshape(P, -1)).astype(np.float16)


def _pack_act(x):
    """(L, D) fp32 -> packed fp16 (L, D): row (j*4+s)*128+p, col dcs*512+c
    holds x[j*512+c, (s*4+dcs)*128+p]."""
    xt = np.ascontiguousarray(np.asarray(x, np.float32).T)   # (D, L)
    a = xt.reshape(NSUB, 4, P, NJ, CH)        # [s, dcs, p, j, c]
    a = a.transpose(3, 0, 2, 1, 4)            # [j, s, p, dcs, c]
    return np.ascontiguousarray(a.reshape(L, D)).astype(np.float16)


def _make_in_maps(Q, K, V, Wq, Wk, Wv, Wo):
    f16 = np.float16
    # fold GQA group sum into Wq: head = g*KV_HEADS + h
    Wq_eff = np.asarray(Wq, np.float32).reshape(D, GROUPS, KV_HEADS, DK).sum(axis=1)
    acts = {}
    for b in range(B):
        acts[b] = {
            "qx": _pack_act(Q[b]),
            "kx": _pack_act(K[b]),
            "vx": _pack_act(V[b]),
        }
    Wk32, Wv32 = np.asarray(Wk, np.float32), np.asarray(Wv, np.float32)
    Wo32 = np.asarray(Wo, np.float32)
    in_maps = []
    for c in range(8):
        b, h = divmod(c, KV_HEADS)
        in_maps.append({
            **acts[b],
            "wq": _pack_w(Wq_eff[:, h, :]),
            "wk": _pack_w(Wk32[:, h * DK:(h + 1) * DK]),
            "wv": _pack_w(Wv32[:, h * DV:(h + 1) * DV]),
            "wo": Wo32[h * DV:(h + 1) * DV, :].astype(f16),
        })
    return in_maps


def _gather(results):
    Y = np.zeros((B, L, D), np.float32)
    for c in range(8):
        Y[c // KV_HEADS] += results[c]["y"].astype(np.float32)
    return Y


def kernel(Q, K, V, Wq, Wk, Wv, Wo):
    nc = _get_nc()
    in_maps = _make_in_maps(Q, K, V, Wq, Wk, Wv, Wo)
    res = bass_utils.run_bass_kernel_spmd(nc, in_maps, core_ids=list(range(8)))
    return _gather(res.results)


def _install_ntff_hook():
    """The agent image's antenv lacks axon_hooks; synthesize it so
    trace=True can reach the NTFF profiler in libaxon_pjrt.so."""
    import types
    import antenv
    if hasattr(antenv, "axon_hooks"):
        return
    mod = types.ModuleType("antenv.axon_hooks")
    _h = [None]
    mod.set_axon_ntff_profile_hook = lambda h: _h.__setitem__(0, h)
    mod.get_axon_ntff_profile_hook = lambda: _h[0]
    sys.modules["antenv.axon_hooks"] = mod
    antenv.axon_hooks = mod
    from trn_agent_boot.trn_boot import _ntff_profile_via_ctypes
    mod.set_axon_ntff_profile_hook(_ntff_profile_via_ctypes("/opt/axon/libaxon_pjrt.so"))


def kernel_traced(Q, K, V, Wq, Wk, Wv, Wo):
    """Like kernel() but profiles; returns (output, BassKernelResults)."""
    _install_ntff_hook()
    nc = _get_nc()
    in_maps = _make_in_maps(Q, K, V, Wq, Wk, Wv, Wo)
    res = bass_utils.run_bass_kernel_spmd(nc, in_maps, core_ids=list(range(8)),
                                          trace=True)
    return _gather(res.results), res
